# revision 12
# baseline (speedup 1.0000x reference)
"""Trainium2 Bass kernel for nn_AttentionModule (gated-SE + global attention pooling GNN).

Math (per reference):
  att = tanh(relu(x@w1+b1)@w2+b2); x2 = (1+att)*x = 2*sigmoid(2*(pre+b2))*x
  mean = segment_mean(x2, batch); tg = tanh(mean @ W)
  coef = sigmoid(sum(x2 * tg[batch], -1)); out = segment_sum(coef[:,None]*x2, batch)

Strategy: data-parallel over graphs (256 graphs/core on 8 cores; batch is sorted so
each core's nodes are contiguous). Per core, two 128-graph windows. Nodes are padded
so each window has a uniform number NBW of 128-node blocks on every core (SPMD: one
program, per-core data). batch is sorted and min graph size > 128, so every 128-node
block touches at most 2 consecutive graphs -> each block writes a *pair* of partial
segment sums into its own block-indexed PSUM columns; a host-built combine matrix
(0/1, per-core data) reduces pairs -> graphs with one matmul. All data-dependent
indexing lives in host-built mask/gather/combine matrices, never in the program.
"""

import math
from contextlib import ExitStack

import numpy as np

P = 128
D = 128
R = 32
G = 2048
NCORES = 8
GPC = G // NCORES       # graphs per core = 256
WG = 128                # graphs per window
NW = GPC // WG          # windows per core = 2

_F32 = np.float32


def _bf16():
    import ml_dtypes
    return ml_dtypes.bfloat16


# ---------------------------------------------------------------- host prep

def _prep(x, batch):
    """Build per-core padded node streams + mask/gather/combine matrices."""
    bf16 = _bf16()
    N = x.shape[0]
    counts = np.bincount(batch, minlength=G).astype(np.int64)
    cum = np.concatenate([[0], np.cumsum(counts)])

    # window node ranges
    win_rng = []  # (core, w) -> (s, e)
    for c in range(NCORES):
        for w in range(NW):
            glo = c * GPC + w * WG
            win_rng.append((int(cum[glo]), int(cum[glo + WG])))
    max_nodes = max(e - s for s, e in win_rng)
    NBW = (max_nodes + P - 1) // P
    NBW = ((NBW + 63) // 64) * 64          # NPAIR == 2*NBW (all pair cols written)
    assert 2 * NBW <= 512, f"window too large: NBW={NBW}"
    NPW = NBW * P
    NPAIR_RAW = 2 * NBW
    NCHK = (NPAIR_RAW + P - 1) // P
    NPAIR = NCHK * P

    xs = np.zeros((NCORES, NW * NPW, D), dtype=bf16)
    m2 = np.zeros((NCORES, NW, P, 2 * NBW), dtype=bf16)
    gm = np.zeros((NCORES, NW, WG, NPAIR), dtype=bf16)
    cb = np.zeros((NCORES, NW, NCHK, P, WG), dtype=_F32)
    ic = np.zeros((NCORES, NW, WG, 1), dtype=_F32)

    wpb_max = 1
    for c in range(NCORES):
        for w in range(NW):
            s, e = win_rng[c * NW + w]
            n = e - s
            glo = c * GPC + w * WG
            xs[c, w * NPW : w * NPW + n] = x[s:e].astype(bf16)
            lid = np.full(NPW, -1, dtype=np.int64)
            lid[:n] = batch[s:e] - glo
            ic[c, w, :, 0] = 1.0 / np.maximum(counts[glo : glo + WG], 1)
            for b in range(NBW):
                ids = lid[b * P : (b + 1) * P]
                uniq = np.unique(ids[ids >= 0])
                wpb_max = max(wpb_max, len(uniq))
                if len(uniq) == 0:
                    gp = [0, 1]
                elif len(uniq) == 1:
                    g0 = int(uniq[0])
                    gp = [g0, g0 + 1 if g0 + 1 < WG else g0 - 1]
                else:
                    gp = [int(uniq[0]), int(uniq[1])]
                for j, gcol in enumerate(gp):
                    sel = ids == gcol
                    if sel.any():
                        m2[c, w, sel, 2 * b + j] = 1.0
                    gm[c, w, gcol, 2 * b + j] = 1.0
                    pair = 2 * b + j
                    cb[c, w, pair // P, pair % P, gcol] = 1.0
    assert wpb_max <= 2, f"block spans {wpb_max} graphs; pair assumption violated"
    return xs, m2, gm, cb, ic, NBW, NPW, NPAIR, NCHK


# ---------------------------------------------------------------- program

def _build(NBW, NPW, NPAIR, NCHK, use_b1=False, use_b2=False):
    import concourse.bass as bass
    import concourse.bacc as bacc
    import concourse.tile as tile
    from concourse import mybir
    from concourse.alu_op_type import AluOpType

    f32 = mybir.dt.float32
    bf = mybir.dt.bfloat16
    AF = mybir.ActivationFunctionType
    NGRP = NBW // 16

    nc = bacc.Bacc()
    xd = nc.dram_tensor("x", [NW * NPW, D], bf, kind="ExternalInput")
    m2d = nc.dram_tensor("m2", [NW, P, 2 * NBW], bf, kind="ExternalInput")
    gmd = nc.dram_tensor("gm", [NW, WG, NPAIR], bf, kind="ExternalInput")
    cbd = nc.dram_tensor("cb", [NW, NCHK, P, WG], f32, kind="ExternalInput")
    icd = nc.dram_tensor("ic", [NW, WG, 1], f32, kind="ExternalInput")
    w1d = nc.dram_tensor("w1", [D, P], bf, kind="ExternalInput")
    w2d = nc.dram_tensor("w2", [P, D], bf, kind="ExternalInput")
    Wd = nc.dram_tensor("Wm", [D, D], f32, kind="ExternalInput")
    b1d = nc.dram_tensor("b1r", [P, 1], f32, kind="ExternalInput")
    b2d = nc.dram_tensor("b2x2", [P, 1], f32, kind="ExternalInput")
    idbd = nc.dram_tensor("idb", [P, P], bf, kind="ExternalInput")
    idfd = nc.dram_tensor("idf", [P, P], f32, kind="ExternalInput")
    outd = nc.dram_tensor("out", [GPC, D], f32, kind="ExternalOutput")

    with tile.TileContext(nc) as tc, ExitStack() as ctx:
        sing = ctx.enter_context(tc.tile_pool(name="sing", bufs=1))
        xtp = ctx.enter_context(tc.tile_pool(name="xtp", bufs=8))
        hsp = ctx.enter_context(tc.tile_pool(name="hsp", bufs=2))
        sgp = ctx.enter_context(tc.tile_pool(name="sgp", bufs=2))
        mkp = ctx.enter_context(tc.tile_pool(name="mkp", bufs=2))
        gbp = ctx.enter_context(tc.tile_pool(name="gbp", bufs=2))
        tgp = ctx.enter_context(tc.tile_pool(name="tgp", bufs=2))
        cbp = ctx.enter_context(tc.tile_pool(name="cbp", bufs=2))
        mds = ctx.enter_context(tc.tile_pool(name="mds", bufs=4))
        ssp = ctx.enter_context(tc.tile_pool(name="ssp", bufs=3))
        big = ctx.enter_context(tc.tile_pool(name="big", bufs=1))
        # psum pools (8 banks): h(2) att(2) xn(2) pair(1) pt(1)
        hpp = ctx.enter_context(tc.tile_pool(name="hpp", bufs=2, space="PSUM"))
        app = ctx.enter_context(tc.tile_pool(name="app", bufs=2, space="PSUM"))
        xnp = ctx.enter_context(tc.tile_pool(name="xnp", bufs=2, space="PSUM"))
        prp = ctx.enter_context(tc.tile_pool(name="prp", bufs=1, space="PSUM"))
        ptp = ctx.enter_context(tc.tile_pool(name="ptp", bufs=1, space="PSUM"))

        w1s = sing.tile([D, P], bf)
        nc.gpsimd.dma_start(out=w1s, in_=w1d[:, :])
        w2s = sing.tile([P, D], bf)
        nc.gpsimd.dma_start(out=w2s, in_=w2d[:, :])
        Ws = sing.tile([D, D], f32)
        nc.gpsimd.dma_start(out=Ws, in_=Wd[:, :])
        b1s = sing.tile([P, 1], f32)
        nc.gpsimd.dma_start(out=b1s, in_=b1d[:, :])
        b2s = sing.tile([P, 1], f32)
        nc.gpsimd.dma_start(out=b2s, in_=b2d[:, :])
        idb = sing.tile([P, P], bf)
        nc.gpsimd.dma_start(out=idb, in_=idbd[:, :])
        idf = sing.tile([P, P], f32)
        nc.gpsimd.dma_start(out=idf, in_=idfd[:, :])

        for w in range(NW):
            x2T = big.tile([P, NPW], bf, tag="x2T")
            x2n = big.tile([P, NPW], bf, tag="x2n")
            cbw0 = cbp.tile([P, NCHK, WG], f32, tag="cb0")
            nc.gpsimd.dma_start(out=cbw0, in_=cbd[w].rearrange("k p g -> p k g"))
            cbw = cbp.tile([P, NCHK, WG], f32, tag="cb1")
            nc.scalar.copy(cbw, cbw0)
            gb0 = gbp.tile([WG, NPAIR], bf, tag="gb0")
            nc.gpsimd.dma_start(out=gb0, in_=gmd[w, :, :])
            gb = gbp.tile([WG, NPAIR], bf, tag="gb1")
            nc.scalar.copy(gb, gb0)
            ics = mds.tile([WG, 1], f32, tag="ic")
            nc.gpsimd.dma_start(out=ics, in_=icd[w, :, :])
            mkb0 = mkp.tile([P, 2 * NBW], bf, tag="mk0")
            nc.gpsimd.dma_start(out=mkb0, in_=m2d[w, :, :])
            mkb = mkp.tile([P, 2 * NBW], bf, tag="mk1")
            nc.scalar.copy(mkb, mkb0)

            # ---------------- phase 1
            pair = prp.tile([P, NPAIR], f32, tag="pair")
            for g in range(NGRP):
                xts = []
                for sb in range(4):
                    b0 = (g * 16 + sb * 4) * P
                    xt = xtp.tile([P, 512], bf, tag="xt")
                    nc.sync.dma_start(
                        out=xt, in_=xd[w * NPW + b0 : w * NPW + b0 + 512, :],
                        transpose=True,
                    )
                    xts.append(xt)
                for sb in range(4):
                    hps = hpp.tile([P, 512], f32, tag="h")
                    nc.tensor.matmul(hps, lhsT=w1s, rhs=xts[sb],
                                     start=True, stop=True)
                    hs = hsp.tile([P, 512], bf, tag="hs")
                    nc.scalar.activation(hs, hps, AF.Relu,
                                         bias=b1s if use_b1 else 0.0)
                    att = app.tile([P, 512], f32, tag="att")
                    nc.tensor.matmul(att, lhsT=w2s, rhs=hs, start=True, stop=True)
                    sg = sgp.tile([P, 512], bf, tag="sg")
                    nc.scalar.activation(sg, att, AF.Sigmoid,
                                         bias=b2s if use_b2 else 0.0, scale=2.0)
                    c0 = (g * 16 + sb * 4) * P
                    nc.vector.scalar_tensor_tensor(
                        out=x2T[:, c0 : c0 + 512], in0=sg, scalar=2.0,
                        in1=xts[sb], op0=AluOpType.mult, op1=AluOpType.mult,
                    )
                for sb in range(4):
                    xnt = xnp.tile([P, 512], f32, tag="xn")
                    c0 = (g * 16 + sb * 4) * P
                    for k in range(4):
                        nc.tensor.matmul(
                            xnt[:, 128 * k : 128 * k + 128],
                            lhsT=x2T[:, c0 + 128 * k : c0 + 128 * k + 128],
                            rhs=idb, start=True, stop=True)
                    nc.scalar.copy(x2n[:, c0 : c0 + 512], xnt)
                for k in range(16):
                    b = g * 16 + k
                    nc.tensor.matmul(pair[:, 2 * b : 2 * b + 2],
                                     lhsT=x2n[:, b * P : b * P + P],
                                     rhs=mkb[:, 2 * b : 2 * b + 2],
                                     start=True, stop=True)

            # ---------------- mid: pairs -> graphs -> mean -> tgT/tg/tgpairT
            sps = mds.tile([P, NPAIR], f32, tag="sps")
            nc.scalar.copy(sps, pair)
            segn = xnp.tile([P, 512], f32, tag="xn")
            for k in range(NCHK):
                tp = ptp.tile([P, 128], f32, tag="pt")
                nc.tensor.matmul(tp, lhsT=sps[:, k * P : (k + 1) * P], rhs=idf,
                                 start=True, stop=True)
                spn = mds.tile([P, 128], f32, tag="spn")
                nc.scalar.copy(spn, tp)
                nc.tensor.matmul(segn[:, :128], lhsT=cbw[:, k, :], rhs=spn,
                                 start=(k == 0), stop=(k == NCHK - 1))
            meann = mds.tile([P, 128], f32, tag="meann")
            nc.vector.tensor_scalar_mul(meann, segn[:, :128], ics)
            tp = ptp.tile([P, 128], f32, tag="pt")
            nc.tensor.matmul(tp, lhsT=meann, rhs=idf, start=True, stop=True)
            meanT = mds.tile([P, 128], f32, tag="meanT")
            nc.scalar.copy(meanT, tp)
            tp2 = ptp.tile([P, 128], f32, tag="pt")
            nc.tensor.matmul(tp2, lhsT=Ws, rhs=meanT, start=True, stop=True)
            tgT = mds.tile([P, 128], bf, tag="tgT")
            nc.scalar.activation(tgT, tp2, AF.Tanh)
            tp3 = ptp.tile([P, 128], f32, tag="pt")
            nc.tensor.matmul(tp3, lhsT=tgT, rhs=idb, start=True, stop=True)
            tgn = mds.tile([P, 128], bf, tag="tgn")
            nc.scalar.copy(tgn, tp3)
            tp4 = xnp.tile([P, 512], f32, tag="xn")
            nc.tensor.matmul(tp4[:, :NPAIR], lhsT=tgn, rhs=gb, start=True, stop=True)
            tgpair = tgp.tile([P, NPAIR], bf)
            nc.scalar.copy(tgpair, tp4[:, :NPAIR])

            # ---------------- phase 2
            opair = prp.tile([P, NPAIR], f32, tag="pair")
            for g in range(NGRP):
                ptt = ptp.tile([P, 32], f32, tag="pt")
                for k in range(16):
                    b = g * 16 + k
                    nc.tensor.matmul(ptt[:, 2 * k : 2 * k + 2],
                                     lhsT=x2T[:, b * P : b * P + P],
                                     rhs=tgpair[:, 2 * b : 2 * b + 2],
                                     start=True, stop=True)
                import concourse.bass as bass_mod
                mslice = mkb[:, 32 * g : 32 * g + 32]
                tmp = ssp.tile([P, 32], f32, tag="tmp")
                nc.vector.tensor_tensor(tmp, ptt, mslice, op=AluOpType.mult)
                sred = ssp.tile([P, 16], f32, tag="sred")
                nc.vector.reduce_sum(sred, tmp.rearrange("p (k t) -> p k t", t=2),
                                     axis=mybir.AxisListType.X)
                coef = ssp.tile([P, 16], f32, tag="coef")
                nc.scalar.activation(coef, sred, AF.Sigmoid)
                cmk = ssp.tile([P, 32], bf, tag="cmk")
                coef_b = bass_mod.AP(
                    tensor=coef.tensor, offset=coef.offset,
                    ap=[list(coef.ap[0]), list(coef.ap[1]), [0, 2]])
                nc.vector.tensor_tensor(
                    cmk.rearrange("p (k t) -> p k t", t=2),
                    mslice.rearrange("p (k t) -> p k t", t=2),
                    coef_b, op=AluOpType.mult)
                for k in range(16):
                    b = g * 16 + k
                    nc.tensor.matmul(opair[:, 2 * b : 2 * b + 2],
                                     lhsT=x2n[:, b * P : b * P + P],
                                     rhs=cmk[:, 2 * k : 2 * k + 2],
                                     start=True, stop=True)

            # ---------------- out combine
            ops = mds.tile([P, NPAIR], f32, tag="sps")
            nc.scalar.copy(ops, opair)
            outn = xnp.tile([P, 512], f32, tag="xn")
            for k in range(NCHK):
                tp = ptp.tile([P, 128], f32, tag="pt")
                nc.tensor.matmul(tp, lhsT=ops[:, k * P : (k + 1) * P], rhs=idf,
                                 start=True, stop=True)
                opn = mds.tile([P, 128], f32, tag="spn")
                nc.scalar.copy(opn, tp)
                nc.tensor.matmul(outn[:, :128], lhsT=cbw[:, k, :], rhs=opn,
                                 start=(k == 0), stop=(k == NCHK - 1))
            outs = mds.tile([P, 128], f32, tag="outs")
            nc.scalar.copy(outs, outn[:, :128])
            nc.gpsimd.dma_start(out=outd[w * WG : (w + 1) * WG, :], in_=outs)

    nc.compile()
    return nc


# ---------------------------------------------------------------- driver

def _run(inputs, trace=False):
    import sys
    if "/opt/trn_rl_repo" not in sys.path:
        sys.path.insert(0, "/opt/trn_rl_repo")
    from concourse.bass_utils import run_bass_kernel_spmd

    bf16 = _bf16()
    x = np.asarray(inputs["x"], _F32)
    batch = np.asarray(inputs["batch"]).astype(np.int64)
    fc_w1 = np.asarray(inputs["fc_w1"], _F32)
    fc_b1 = np.asarray(inputs["fc_b1"], _F32)
    fc_w2 = np.asarray(inputs["fc_w2"], _F32)
    fc_b2 = np.asarray(inputs["fc_b2"], _F32)
    W = np.asarray(inputs["W"], _F32)

    xs, m2, gm, cb, ic, NBW, NPW, NPAIR, NCHK = _prep(x, batch)
    nc = _build(NBW, NPW, NPAIR, NCHK, use_b1=bool(np.abs(fc_b1).max() > 0), use_b2=bool(np.abs(fc_b2).max() > 0))

    w1p = np.zeros((P, P), dtype=bf16); w1p[:, :R] = fc_w1.astype(bf16)
    w2p = np.zeros((P, P), dtype=bf16); w2p[:R, :] = fc_w2.astype(bf16)
    b1r = np.concatenate([fc_b1, np.zeros(P - R, np.float32)]).reshape(P, 1).astype(_F32)
    b2x2 = (2.0 * fc_b2).reshape(P, 1).astype(_F32)
    idb = np.eye(P, dtype=_F32).astype(bf16)
    idf = np.eye(P, dtype=_F32)
    in_maps = []
    for c in range(NCORES):
        in_maps.append({
            "x": xs[c], "m2": m2[c], "gm": gm[c], "cb": cb[c], "ic": ic[c],
            "w1": w1p, "w2": w2p,
            "Wm": W, "b1r": b1r, "b2x2": b2x2, "idb": idb, "idf": idf,
        })
    res = run_bass_kernel_spmd(nc, in_maps, core_ids=list(range(NCORES)),
                               trace=trace)
    out = np.concatenate([np.asarray(r["out"], _F32) for r in res.results], axis=0)
    return out, res


def kernel(**inputs) -> np.ndarray:
    out, _ = _run(inputs, trace=False)
    return out


# ------------------------------------------------- bench (timing) harness

def _bench(inputs, iters=24):
    """Return (out, per_call_ns). Steady-state throughput via async enqueue."""
    import sys, time
    if "/opt/trn_rl_repo" not in sys.path:
        sys.path.insert(0, "/opt/trn_rl_repo")
    import jax
    import numpy as np2
    from jax.experimental.shard_map import shard_map
    from jax.sharding import Mesh, PartitionSpec
    from concourse import bass2jax, mybir
    from concourse.bass2jax import _bass_exec_p, partition_id_tensor

    bass2jax.install_neuronx_cc_hook()
    bf16 = _bf16()
    x = np.asarray(inputs["x"], _F32)
    batch = np.asarray(inputs["batch"]).astype(np.int64)
    fc_w1 = np.asarray(inputs["fc_w1"], _F32)
    fc_b1 = np.asarray(inputs["fc_b1"], _F32)
    fc_w2 = np.asarray(inputs["fc_w2"], _F32)
    fc_b2 = np.asarray(inputs["fc_b2"], _F32)
    W = np.asarray(inputs["W"], _F32)
    xs, m2, gm, cb, ic, NBW, NPW, NPAIR, NCHK = _prep(x, batch)
    nc = _build(NBW, NPW, NPAIR, NCHK,
                use_b1=bool(np.abs(fc_b1).max() > 0),
                use_b2=bool(np.abs(fc_b2).max() > 0))
    w1p = np.zeros((P, P), dtype=bf16); w1p[:, :R] = fc_w1.astype(bf16)
    w2p = np.zeros((P, P), dtype=bf16); w2p[:R, :] = fc_w2.astype(bf16)
    b1r = np.concatenate([fc_b1, np.zeros(P - R, np.float32)]).reshape(P, 1)
    b2x2 = (2.0 * fc_b2).reshape(P, 1).astype(_F32)
    idb = np.eye(P, dtype=_F32).astype(bf16)
    idf = np.eye(P, dtype=_F32)
    in_maps = [{
        "x": xs[c], "m2": m2[c], "gm": gm[c], "cb": cb[c], "ic": ic[c],
        "w1": w1p, "w2": w2p, "Wm": W, "b1r": b1r.astype(_F32), "b2x2": b2x2,
        "idb": idb, "idf": idf} for c in range(NCORES)]

    in_names, out_names, out_avals, zero_outs = [], [], [], []
    for alloc in nc.m.functions[0].allocations:
        if not isinstance(alloc, mybir.MemoryLocationSet):
            continue
        name = alloc.memorylocations[0].name
        if alloc.kind == "ExternalInput":
            if nc.partition_id_tensor is None or name != nc.partition_id_tensor.name:
                in_names.append(name)
        elif alloc.kind == "ExternalOutput":
            shape = tuple(alloc.tensor_shape)
            dtype = mybir.dt.np(alloc.dtype)
            out_names.append(name)
            out_avals.append(jax.core.ShapedArray(shape, dtype))
            zero_outs.append(np.zeros(shape, dtype))
    n_params = len(in_names)
    all_names = in_names + out_names
    pname = nc.partition_id_tensor.name if nc.partition_id_tensor else None
    if pname is not None:
        all_names.append(pname)

    def _body(*args):
        operands = list(args)
        if pname is not None:
            operands.append(partition_id_tensor())
        return tuple(_bass_exec_p.bind(
            *operands, out_avals=tuple(out_avals), in_names=tuple(all_names),
            out_names=tuple(out_names), lowering_input_output_aliases=(),
            sim_require_finite=True, sim_require_nnan=True, nc=nc))

    devices = jax.devices()[:NCORES]
    mesh = Mesh(np.asarray(devices), ("core",))
    nio = n_params + len(out_names)
    fn = jax.jit(shard_map(_body, mesh=mesh,
                           in_specs=(PartitionSpec("core"),) * nio,
                           out_specs=(PartitionSpec("core"),) * len(out_names),
                           check_rep=False), keep_unused=True)
    concat_in = [np.concatenate([np.asarray(in_maps[c][nm])[None] for c in range(NCORES)],
                                axis=0).reshape(-1, *np.asarray(in_maps[0][nm]).shape[1:])
                 for nm in in_names]
    concat_zero = [np.concatenate([z[None]] * NCORES, axis=0).reshape(-1, *z.shape[1:])
                   for z in zero_outs]
    dev_in = [jax.device_put(a) for a in concat_in + concat_zero]
    outs = fn(*dev_in)
    jax.block_until_ready(outs)
    t0 = time.perf_counter()
    outs = fn(*dev_in)
    jax.block_until_ready(outs)
    t1 = time.perf_counter()
    one = t1 - t0
    t0 = time.perf_counter()
    last = None
    for _ in range(iters):
        last = fn(*dev_in)
    jax.block_until_ready(last)
    t2 = time.perf_counter()
    per = (t2 - t0) / iters
    out_full = np.concatenate(
        [np.asarray(outs[0]).reshape(NCORES, GPC, D)[c] for c in range(NCORES)], axis=0)
    return out_full.astype(np.float32), per * 1e9, one * 1e9


# revision 26
# speedup vs baseline: 167.8982x; 167.8982x over previous
"""Trainium2 Bass kernel for nn_AttentionModule (gated-SE + global attention pooling GNN).

Math (per reference):
  att = tanh(relu(x@w1+b1)@w2+b2); x2 = (1+att)*x = 2*sigmoid(2*(pre+b2))*x
  mean = segment_mean(x2, batch); tg = tanh(mean @ W)
  coef = sigmoid(sum(x2 * tg[batch], -1)); out = segment_sum(coef[:,None]*x2, batch)

Strategy: data-parallel over graphs (256 graphs/core on 8 cores; batch is sorted so
each core's nodes are contiguous). Per core, four 64-graph windows, software-
pipelined (p1(w+1) emitted before p2(w)). Nodes padded so each window has a uniform
block count NBW on every core (SPMD: one program, per-core data). Every 128-node
block touches at most 2 graphs (min graph size > 128) -> per-block *pair* partial
segment sums in block-indexed PSUM columns; host-built 0/1 combine matrices reduce
pairs -> graphs. All data-dependent indexing lives in host-built mask/gather/combine
matrices, never in the program. The device stores x2' = sigmoid(2 z)*x (= x2/2) in
both layouts (x2T via DMA-transposed input, x2n via bf16 PE transposes); the factor
2 is folded into inv_counts, the coef sigmoid scale, and a final host-side doubling.
"""

from contextlib import ExitStack

import numpy as np

P = 128
D = 128
R = 32
G = 2048
NCORES = 8
GPC = G // NCORES       # graphs per core = 256
WG = 64                 # graphs per window
NW = GPC // WG          # windows per core = 4

_F32 = np.float32


def _bf16():
    import ml_dtypes
    return ml_dtypes.bfloat16


# ---------------------------------------------------------------- host prep

def _prep(x, batch):
    """Build per-core padded node streams + mask/gather/combine matrices."""
    bf16 = _bf16()
    counts = np.bincount(batch, minlength=G).astype(np.int64)
    cum = np.concatenate([[0], np.cumsum(counts)])

    win_rng = []  # (core, w) -> (s, e)
    for c in range(NCORES):
        for w in range(NW):
            glo = c * GPC + w * WG
            win_rng.append((int(cum[glo]), int(cum[glo + WG])))
    max_nodes = max(e - s for s, e in win_rng)
    NBW = (max_nodes + P - 1) // P
    NBW = ((NBW + 63) // 64) * 64          # NPAIR == 2*NBW (all pair cols written)
    assert 2 * NBW <= 512, f"window too large: NBW={NBW}"
    NPW = NBW * P
    NPAIR = 2 * NBW
    NCHK = NPAIR // P

    xs = np.zeros((NCORES, NW * NPW, D), dtype=bf16)
    m2 = np.zeros((NCORES, NW, P, NPAIR), dtype=bf16)
    gm = np.zeros((NCORES, NW, WG, NPAIR), dtype=bf16)
    cb = np.zeros((NCORES, NW, NCHK, P, WG), dtype=_F32)
    ic = np.zeros((NCORES, NW, WG, 1), dtype=_F32)

    wpb_max = 1
    for c in range(NCORES):
        for w in range(NW):
            s, e = win_rng[c * NW + w]
            n = e - s
            glo = c * GPC + w * WG
            xs[c, w * NPW : w * NPW + n] = x[s:e].astype(bf16)
            lid = np.full(NPW, -1, dtype=np.int64)
            lid[:n] = batch[s:e] - glo
            # factor 2 of x2 = 2*x2' folded here (mean needs true x2)
            ic[c, w, :, 0] = 2.0 / np.maximum(counts[glo : glo + WG], 1)
            for b in range(NBW):
                ids = lid[b * P : (b + 1) * P]
                uniq = np.unique(ids[ids >= 0])
                wpb_max = max(wpb_max, len(uniq))
                if len(uniq) == 0:
                    gp = [0, 1]
                elif len(uniq) == 1:
                    g0 = int(uniq[0])
                    gp = [g0, g0 + 1 if g0 + 1 < WG else g0 - 1]
                else:
                    gp = [int(uniq[0]), int(uniq[1])]
                for j, gcol in enumerate(gp):
                    sel = ids == gcol
                    if sel.any():
                        m2[c, w, sel, 2 * b + j] = 1.0
                    gm[c, w, gcol, 2 * b + j] = 1.0
                    pr = 2 * b + j
                    cb[c, w, pr // P, pr % P, gcol] = 1.0
    assert wpb_max <= 2, f"block spans {wpb_max} graphs; pair assumption violated"
    return xs, m2, gm, cb, ic, NBW, NPW, NPAIR, NCHK


# ---------------------------------------------------------------- program

def _build(NBW, NPW, NPAIR, NCHK, use_b1=False, use_b2=False):
    import concourse.bass as bass_mod
    import concourse.bacc as bacc
    import concourse.tile as tile
    from concourse import mybir
    from concourse.alu_op_type import AluOpType

    f32 = mybir.dt.float32
    bf = mybir.dt.bfloat16
    AF = mybir.ActivationFunctionType
    NGRP = NBW // 16

    nc = bacc.Bacc()
    xd = nc.dram_tensor("x", [NW * NPW, D], bf, kind="ExternalInput")
    m2d = nc.dram_tensor("m2", [NW, P, NPAIR], bf, kind="ExternalInput")
    gmd = nc.dram_tensor("gm", [NW, WG, NPAIR], bf, kind="ExternalInput")
    cbd = nc.dram_tensor("cb", [NW, NCHK, P, WG], f32, kind="ExternalInput")
    icd = nc.dram_tensor("ic", [NW, WG, 1], f32, kind="ExternalInput")
    w1d = nc.dram_tensor("w1", [D, R], bf, kind="ExternalInput")
    w2d = nc.dram_tensor("w2", [P, D], bf, kind="ExternalInput")
    Wd = nc.dram_tensor("Wm", [D, D], f32, kind="ExternalInput")
    b1d = nc.dram_tensor("b1r", [P, 1], f32, kind="ExternalInput")
    b2d = nc.dram_tensor("b2x2", [P, 1], f32, kind="ExternalInput")
    idbd = nc.dram_tensor("idb", [P, P], bf, kind="ExternalInput")
    idfd = nc.dram_tensor("idf", [P, P], f32, kind="ExternalInput")
    outd = nc.dram_tensor("out", [GPC, D], f32, kind="ExternalOutput")

    with tile.TileContext(nc) as tc, ExitStack() as ctx:
        sing = ctx.enter_context(tc.tile_pool(name="sing", bufs=1))
        xtp = ctx.enter_context(tc.tile_pool(name="xtp", bufs=4))
        hsp = ctx.enter_context(tc.tile_pool(name="hsp", bufs=3))
        sgp = ctx.enter_context(tc.tile_pool(name="sgp", bufs=3))
        mkp = ctx.enter_context(tc.tile_pool(name="mkp", bufs=2))
        gbp = ctx.enter_context(tc.tile_pool(name="gbp", bufs=2))
        tgp = ctx.enter_context(tc.tile_pool(name="tgp", bufs=2))
        cbp = ctx.enter_context(tc.tile_pool(name="cbp", bufs=2))
        mds = ctx.enter_context(tc.tile_pool(name="mds", bufs=4))
        ssp = ctx.enter_context(tc.tile_pool(name="ssp", bufs=4))
        big = ctx.enter_context(tc.tile_pool(name="big", bufs=2))
        # psum pools, 8 banks total: h(1) att(2) xn(2) pair(2) pt(1)
        hpp = ctx.enter_context(tc.tile_pool(name="hpp", bufs=1, space="PSUM"))
        app = ctx.enter_context(tc.tile_pool(name="app", bufs=1, space="PSUM"))
        xnp = ctx.enter_context(tc.tile_pool(name="xnp", bufs=2, space="PSUM"))
        prp = ctx.enter_context(tc.tile_pool(name="prp", bufs=2, space="PSUM"))
        ptp = ctx.enter_context(tc.tile_pool(name="ptp", bufs=1, space="PSUM"))

        w1s = sing.tile([D, R], bf)
        nc.gpsimd.dma_start(out=w1s, in_=w1d[:, :])
        w2s = sing.tile([P, D], bf)
        nc.gpsimd.dma_start(out=w2s, in_=w2d[:, :])
        Ws = sing.tile([D, D], f32)
        nc.gpsimd.dma_start(out=Ws, in_=Wd[:, :])
        b1s = sing.tile([P, 1], f32)
        nc.gpsimd.dma_start(out=b1s, in_=b1d[:, :])
        b2s = sing.tile([P, 1], f32)
        nc.gpsimd.dma_start(out=b2s, in_=b2d[:, :])
        idb = sing.tile([P, P], bf)
        nc.gpsimd.dma_start(out=idb, in_=idbd[:, :])
        idf = sing.tile([P, P], f32)
        nc.gpsimd.dma_start(out=idf, in_=idfd[:, :])

        st = {}

        def emit_p1(w):
            s = {}
            s["x2T"] = big.tile([P, NPW], bf, tag="x2T", name="x2T")
            s["x2n"] = big.tile([P, NPW], bf, tag="x2n", name="x2n")
            s["cbw"] = cbp.tile([P, NCHK, WG], f32, tag="cb", name="cbw")
            nc.gpsimd.dma_start(out=s["cbw"], in_=cbd[w].rearrange("k p g -> p k g"))
            s["gb"] = gbp.tile([WG, NPAIR], bf, tag="gb", name="gb")
            nc.gpsimd.dma_start(out=s["gb"], in_=gmd[w, :, :])
            s["ics"] = mds.tile([WG, 1], f32, tag="ic", name="ics")
            nc.gpsimd.dma_start(out=s["ics"], in_=icd[w, :, :])
            s["mkb"] = mkp.tile([P, NPAIR], bf, tag="mk", name="mkb")
            nc.gpsimd.dma_start(out=s["mkb"], in_=m2d[w, :, :])
            x2T, x2n, mkb = s["x2T"], s["x2n"], s["mkb"]
            pair = prp.tile([P, NPAIR], f32, tag="pair")
            s["pair"] = pair
            for g in range(NGRP):
                b0 = g * 16 * P
                xt = xtp.tile([P, 2048], bf, tag="xt")
                nc.sync.dma_start(
                    out=xt, in_=xd[w * NPW + b0 : w * NPW + b0 + 2048, :],
                    transpose=True,
                )
                xts = [xt[:, 512 * sb : 512 * sb + 512] for sb in range(4)]
                hps = hpp.tile([P, 512], f32, tag="h")
                for sb in range(4):
                    nc.tensor.matmul(hps[32 * sb : 32 * sb + 32, :], lhsT=w1s,
                                     rhs=xts[sb], start=True, stop=True,
                                     tile_position=(0, 32 * sb))
                hs = hsp.tile([P, 512], bf, tag="hs")
                nc.scalar.activation(hs, hps, AF.Relu,
                                     bias=b1s if use_b1 else 0.0)
                for half in range(2):
                    att = app.tile([P, 1024], f32, tag="att")
                    for s2 in range(2):
                        sb = half * 2 + s2
                        nc.tensor.matmul(att[:, 512 * s2 : 512 * s2 + 512],
                                         lhsT=w2s[32 * sb : 32 * sb + 32, :],
                                         rhs=hs[32 * sb : 32 * sb + 32, :],
                                         start=True, stop=True,
                                         tile_position=(32 * sb, 0))
                    sg = sgp.tile([P, 1024], bf, tag="sg")
                    nc.scalar.activation(sg, att, AF.Sigmoid,
                                         bias=b2s if use_b2 else 0.0, scale=2.0)
                    for s2 in range(2):
                        sb = half * 2 + s2
                        c0 = (g * 16 + sb * 4) * P
                        nc.vector.tensor_tensor(
                            x2T[:, c0 : c0 + 512],
                            sg[:, 512 * s2 : 512 * s2 + 512],
                            xts[sb], op=AluOpType.mult,
                        )
                for hf in range(2):
                    xnt = xnp.tile([P, 1024], bf, tag="xn")
                    c0 = (g * 16 + hf * 8) * P
                    for k in range(8):
                        nc.tensor.transpose(
                            xnt[:, 128 * k : 128 * k + 128],
                            x2T[:, c0 + 128 * k : c0 + 128 * k + 128],
                            idb)
                    nc.vector.tensor_copy(x2n[:, c0 : c0 + 1024], xnt)
                for k in range(16):
                    b = g * 16 + k
                    nc.tensor.matmul(pair[:, 2 * b : 2 * b + 2],
                                     lhsT=x2n[:, b * P : b * P + P],
                                     rhs=mkb[:, 2 * b : 2 * b + 2],
                                     start=True, stop=True)
            st[w] = s

        def emit_mid(w):
            s = st[w]
            cbw, gb, ics, pair = s["cbw"], s["gb"], s["ics"], s["pair"]
            sps = mds.tile([P, NPAIR], f32, tag="sps")
            nc.vector.tensor_copy(sps, pair)
            segn = xnp.tile([P, 512], f32, tag="xn")
            for k in range(NCHK):
                tp = ptp.tile([P, 128], f32, tag="pt")
                nc.tensor.matmul(tp, lhsT=sps[:, k * P : (k + 1) * P], rhs=idf,
                                 start=True, stop=True)
                spn = mds.tile([P, 128], f32, tag="spn")
                nc.vector.tensor_copy(spn, tp)
                nc.tensor.matmul(segn[:WG, :128], lhsT=cbw[:, k, :], rhs=spn,
                                 start=(k == 0), stop=(k == NCHK - 1))
            meann = mds.tile([WG, 128], f32, tag="meann")
            nc.vector.tensor_scalar_mul(meann, segn[:WG, :128], ics)
            tp = ptp.tile([P, 128], f32, tag="pt")
            nc.tensor.matmul(tp[:, :WG], lhsT=meann, rhs=idf[:WG, :WG],
                             start=True, stop=True)
            meanT = mds.tile([P, WG], f32, tag="meanT")
            nc.scalar.copy(meanT, tp[:, :WG])
            tp2 = ptp.tile([P, 128], f32, tag="pt")
            nc.tensor.matmul(tp2[:, :WG], lhsT=Ws, rhs=meanT, start=True, stop=True)
            tgT = mds.tile([P, WG], bf, tag="tgT")
            nc.scalar.activation(tgT, tp2[:, :WG], AF.Tanh)
            tp3 = ptp.tile([P, 128], f32, tag="pt")
            nc.tensor.matmul(tp3[:WG, :], lhsT=tgT, rhs=idb, start=True, stop=True)
            tgn = mds.tile([WG, 128], bf, tag="tgn")
            nc.scalar.copy(tgn, tp3[:WG, :])
            tp4 = xnp.tile([P, 512], f32, tag="xn")
            nc.tensor.matmul(tp4[:, :NPAIR], lhsT=tgn, rhs=gb, start=True, stop=True)
            tgpair = tgp.tile([P, NPAIR], bf)
            nc.scalar.copy(tgpair, tp4[:, :NPAIR])
            s["tgpair"] = tgpair

        def emit_p2(w):
            s = st[w]
            x2T, x2n, mkb, cbw, tgpair = (s["x2T"], s["x2n"], s["mkb"],
                                          s["cbw"], s["tgpair"])
            opair = prp.tile([P, NPAIR], f32, tag="pair")
            for g4 in range(NGRP // 4):
                bb = g4 * 64          # first block of this 4-group super
                ptt = ptp.tile([P, 128], f32, tag="pt")
                for k in range(64):
                    b = bb + k
                    nc.tensor.matmul(ptt[:, 2 * k : 2 * k + 2],
                                     lhsT=x2T[:, b * P : b * P + P],
                                     rhs=tgpair[:, 2 * b : 2 * b + 2],
                                     start=True, stop=True)
                tmp = ssp.tile([P, 128], f32, tag="tmp")
                nc.vector.tensor_tensor(tmp, ptt, mkb[:, 2 * bb : 2 * bb + 128],
                                        op=AluOpType.mult)
                sred = ssp.tile([P, 64], f32, tag="sred")
                nc.vector.reduce_sum(sred, tmp.rearrange("p (k t) -> p k t", t=2),
                                     axis=mybir.AxisListType.X)
                coef = ssp.tile([P, 64], f32, tag="coef")
                nc.scalar.activation(coef, sred, AF.Sigmoid, scale=2.0)
                cmk = ssp.tile([P, 128], bf, tag="cmk")
                coef_b = bass_mod.AP(
                    tensor=coef.tensor, offset=coef.offset,
                    ap=[list(coef.ap[0]), [list(coef.ap[1])[0], 64], [0, 2]])
                nc.vector.tensor_tensor(
                    cmk.rearrange("p (k t) -> p k t", t=2),
                    mkb[:, 2 * bb : 2 * bb + 128].rearrange("p (k t) -> p k t", t=2),
                    coef_b, op=AluOpType.mult)
                for k in range(64):
                    b = bb + k
                    nc.tensor.matmul(opair[:, 2 * b : 2 * b + 2],
                                     lhsT=x2n[:, b * P : b * P + P],
                                     rhs=cmk[:, 2 * k : 2 * k + 2],
                                     start=True, stop=True)
            ops = mds.tile([P, NPAIR], f32, tag="sps")
            nc.vector.tensor_copy(ops, opair)
            outn = xnp.tile([P, 512], f32, tag="xn")
            for k in range(NCHK):
                tp = ptp.tile([P, 128], f32, tag="pt")
                nc.tensor.matmul(tp, lhsT=ops[:, k * P : (k + 1) * P], rhs=idf,
                                 start=True, stop=True)
                opn = mds.tile([P, 128], f32, tag="spn")
                nc.vector.tensor_copy(opn, tp)
                nc.tensor.matmul(outn[:WG, :128], lhsT=cbw[:, k, :], rhs=opn,
                                 start=(k == 0), stop=(k == NCHK - 1))
            outs = mds.tile([WG, 128], f32, tag="outs")
            nc.scalar.copy(outs, outn[:WG, :128])
            nc.gpsimd.dma_start(out=outd[w * WG : (w + 1) * WG, :], in_=outs)
            del st[w]

        for w in range(NW):
            emit_p1(w)
            if w > 0:
                emit_p2(w - 1)
            emit_mid(w)
        emit_p2(NW - 1)

    nc.compile()
    return nc


# ---------------------------------------------------------------- driver

def _make_in_maps(inputs):
    bf16 = _bf16()
    x = np.asarray(inputs["x"], _F32)
    batch = np.asarray(inputs["batch"]).astype(np.int64)
    fc_w1 = np.asarray(inputs["fc_w1"], _F32)
    fc_b1 = np.asarray(inputs["fc_b1"], _F32)
    fc_w2 = np.asarray(inputs["fc_w2"], _F32)
    fc_b2 = np.asarray(inputs["fc_b2"], _F32)
    W = np.asarray(inputs["W"], _F32)

    xs, m2, gm, cb, ic, NBW, NPW, NPAIR, NCHK = _prep(x, batch)
    w1p = fc_w1.astype(bf16)
    w2p = np.tile(fc_w2, (4, 1)).astype(bf16)
    b1r = np.tile(fc_b1, 4).reshape(P, 1).astype(_F32)
    b2x2 = (2.0 * fc_b2).reshape(P, 1).astype(_F32)
    idb = np.eye(P, dtype=_F32).astype(bf16)
    idf = np.eye(P, dtype=_F32)
    in_maps = []
    for c in range(NCORES):
        in_maps.append({
            "x": xs[c], "m2": m2[c], "gm": gm[c], "cb": cb[c], "ic": ic[c],
            "w1": w1p, "w2": w2p, "Wm": W, "b1r": b1r, "b2x2": b2x2,
            "idb": idb, "idf": idf,
        })
    dims = (NBW, NPW, NPAIR, NCHK)
    flags = (bool(np.abs(fc_b1).max() > 0), bool(np.abs(fc_b2).max() > 0))
    return in_maps, dims, flags


def _run(inputs, trace=False):
    import sys
    if "/opt/trn_rl_repo" not in sys.path:
        sys.path.insert(0, "/opt/trn_rl_repo")
    from concourse.bass_utils import run_bass_kernel_spmd

    in_maps, (NBW, NPW, NPAIR, NCHK), (use_b1, use_b2) = _make_in_maps(inputs)
    nc = _build(NBW, NPW, NPAIR, NCHK, use_b1=use_b1, use_b2=use_b2)
    res = run_bass_kernel_spmd(nc, in_maps, core_ids=list(range(NCORES)),
                               trace=trace)
    out = 2.0 * np.concatenate(
        [np.asarray(r["out"], _F32) for r in res.results], axis=0)
    return out.astype(np.float32), res


def kernel(**inputs) -> np.ndarray:
    out, _ = _run(inputs, trace=False)
    return out


# ------------------------------------------------- bench (timing) harness

def _bench(inputs, iters=24):
    """Return (out, per_call_ns, single_ns) via steady-state async enqueue."""
    import sys, time
    if "/opt/trn_rl_repo" not in sys.path:
        sys.path.insert(0, "/opt/trn_rl_repo")
    import jax
    from jax.experimental.shard_map import shard_map
    from jax.sharding import Mesh, PartitionSpec
    from concourse import bass2jax, mybir
    from concourse.bass2jax import _bass_exec_p, partition_id_tensor

    bass2jax.install_neuronx_cc_hook()
    in_maps, (NBW, NPW, NPAIR, NCHK), (use_b1, use_b2) = _make_in_maps(inputs)
    nc = _build(NBW, NPW, NPAIR, NCHK, use_b1=use_b1, use_b2=use_b2)

    in_names, out_names, out_avals, zero_outs = [], [], [], []
    for alloc in nc.m.functions[0].allocations:
        if not isinstance(alloc, mybir.MemoryLocationSet):
            continue
        name = alloc.memorylocations[0].name
        if alloc.kind == "ExternalInput":
            if nc.partition_id_tensor is None or name != nc.partition_id_tensor.name:
                in_names.append(name)
        elif alloc.kind == "ExternalOutput":
            shape = tuple(alloc.tensor_shape)
            dtype = mybir.dt.np(alloc.dtype)
            out_names.append(name)
            out_avals.append(jax.core.ShapedArray(shape, dtype))
            zero_outs.append(np.zeros(shape, dtype))
    n_params = len(in_names)
    all_names = list(in_names) + out_names
    pname = nc.partition_id_tensor.name if nc.partition_id_tensor else None
    if pname is not None:
        all_names.append(pname)

    def _body(*args):
        operands = list(args)
        if pname is not None:
            operands.append(partition_id_tensor())
        return tuple(_bass_exec_p.bind(
            *operands, out_avals=tuple(out_avals), in_names=tuple(all_names),
            out_names=tuple(out_names), lowering_input_output_aliases=(),
            sim_require_finite=True, sim_require_nnan=True, nc=nc))

    devices = jax.devices()[:NCORES]
    mesh = Mesh(np.asarray(devices), ("core",))
    nio = n_params + len(out_names)
    fn = jax.jit(shard_map(_body, mesh=mesh,
                           in_specs=(PartitionSpec("core"),) * nio,
                           out_specs=(PartitionSpec("core"),) * len(out_names),
                           check_rep=False), keep_unused=True)
    concat_in = [np.concatenate([np.asarray(in_maps[c][nm])[None]
                                 for c in range(NCORES)], axis=0)
                 .reshape(-1, *np.asarray(in_maps[0][nm]).shape[1:])
                 for nm in in_names]
    concat_zero = [np.concatenate([z[None]] * NCORES, axis=0)
                   .reshape(-1, *z.shape[1:]) for z in zero_outs]
    dev_in = [jax.device_put(a) for a in concat_in + concat_zero]
    outs = fn(*dev_in)
    jax.block_until_ready(outs)
    t0 = time.perf_counter()
    outs = fn(*dev_in)
    jax.block_until_ready(outs)
    one = time.perf_counter() - t0
    t0 = time.perf_counter()
    last = None
    for _ in range(iters):
        last = fn(*dev_in)
    jax.block_until_ready(last)
    per = (time.perf_counter() - t0) / iters
    out_full = 2.0 * np.concatenate(
        [np.asarray(outs[0]).reshape(NCORES, GPC, D)[c] for c in range(NCORES)],
        axis=0)
    return out_full.astype(np.float32), per * 1e9, one * 1e9
